# revision 12
# baseline (speedup 1.0000x reference)
"""Trainium2 Bass kernel for ViT attention with SAVE spatial augmentation.

Problem shapes: B=64, N=197, C=768, H=12 heads, D=64, L=196 patches, W=72.
Sharding: data-parallel over batch across 8 NeuronCores (8 batch items/core).

Per-core pipeline (all fp32 matmuls on the PE):
  phase T: transpose x into xT [c-on-partitions, tok] via PE transposes
  phase R: build per-head mixing matrix trans_full^T in SBUF, layout
           [k partitions, q, h] (identity + cls handled via host-side
           augmentation of the table with an indicator row)
  per-b loop: QKV matmuls (q,v in [tok,d] form; k in [d,tok] form) ->
           SAVE (q'^T = q_raw^T-contracted with transT, gives [d,q]) ->
           scores S^T[kt,qt] -> exp(scale*S) on ACT -> PV with
           ones-augmented V (unnormalized out + softmax denominator in one
           accumulation) -> normalize (reciprocal + K=1 replication matmul)
           -> output projection.
"""

import numpy as np

import concourse.bass as bass
import concourse.mybir as mybir
import concourse.tile as tile
from concourse import bacc
from concourse.bass_utils import run_bass_kernel_spmd

F32 = mybir.dt.float32

B, N, C, H = 64, 197, 768, 12
D = C // H            # 64
L = N - 1             # 196
W = 72
WA = W + 1            # augmented table rows (extra identity-indicator row)
NCORES = 8
NB = B // NCORES      # batch items per core
CK = C // 128         # 6 c-chunks
SCALE = D ** -0.5

# k-chunk split of the 197 tokens for partition-dim tiling
KT0, KT1 = 128, N - 128     # 128, 69
# kp split of the 196 patch rows so that k=1+kp aligns with the 128-row tile
KPA, KPB = 127, L - 127     # 127, 69


def _body(tc, io, nb):
    nc = tc.nc
    tok = nb * N
    n_mo = (tok + 127) // 128          # full-128 chunks for xT transposes
    QP_SLAB = 14                       # qp values per tabT slab
    N_SLAB = L // QP_SLAB              # 14

    with tile.ExitStack() as ctx:
        persist = ctx.enter_context(tc.tile_pool(name="persist", bufs=1))
        work1 = ctx.enter_context(tc.tile_pool(name="work1", bufs=1))
        work2 = ctx.enter_context(tc.tile_pool(name="work2", bufs=2))
        work3 = ctx.enter_context(tc.tile_pool(name="work3", bufs=3))
        psmall = ctx.enter_context(tc.tile_pool(name="psmall", bufs=4, space="PSUM"))
        pbig = ctx.enter_context(tc.tile_pool(name="pbig", bufs=2, space="PSUM"))

        # ---------------- constants ----------------
        ident = persist.tile([128, 128], F32)
        nc.sync.dma_start(ident[:], io["ident"])
        params_sb = persist.tile([WA, H], F32)
        nc.sync.dma_start(params_sb[:], io["params_aug"])
        w_sb = persist.tile([128, CK, 3 * C], F32)
        nc.sync.dma_start(w_sb[:], io["qkv_w"].rearrange("(ko p) n -> p ko n", p=128))
        pw_sb = persist.tile([128, CK, C], F32)
        nc.sync.dma_start(pw_sb[:], io["proj_w"].rearrange("(ko p) n -> p ko n", p=128))
        qkvb_rep = persist.tile([128, 3 * C], F32)
        nc.sync.dma_start(qkvb_rep[:], io["qkv_b"][None, :].to_broadcast((128, 3 * C)))
        projb_rep = persist.tile([128, C], F32)
        nc.sync.dma_start(projb_rep[:], io["proj_b"][None, :].to_broadcast((128, C)))
        qkvb_k = persist.tile([128, CK], F32)
        nc.sync.dma_start(
            qkvb_k[:], io["qkv_b"][C:2 * C].rearrange("(ko p) -> p ko", p=128)
        )
        ones_sb = persist.tile([1, D], F32)
        nc.vector.memset(ones_sb[:], 1.0)

        # ---------------- phase R: trans_full^T build ----------------
        TR_lo = persist.tile([128, N, H], F32)   # k in 0:128
        TR_hi = persist.tile([KT1, N, H], F32)   # k in 128:197
        nc.gpsimd.memset(TR_lo[:], 0.0)
        nc.gpsimd.memset(TR_hi[:], 0.0)
        nc.vector.memset(TR_lo[0:1, 0:1, :], 1.0)   # cls: TF[0,0]=1

        # tabT_aug layout: [WA, L, N] per-qp blocks of width 197; column 0 is a
        # zero pad for the cls row so chunk A yields k rows 0:128 at base 0.
        with tc.tile_pool(name="slabs", bufs=2) as slabs:
            for s in range(N_SLAB):
                slab = slabs.tile([WA, QP_SLAB * N], F32, tag="tabT")
                nc.sync.dma_start(
                    slab[:], io["tabT_aug"][:, s * QP_SLAB * N:(s + 1) * QP_SLAB * N]
                )
                # groups of 4 qp -> one psum bank each for A(128-part) and B(69)
                for g0 in range(0, QP_SLAB, 4):
                    glen = min(4, QP_SLAB - g0)
                    psA = psmall.tile([128, 512], F32, tag="ps")
                    psB = psmall.tile([128, 512], F32, tag="ps")
                    for j in range(glen):
                        moff = (g0 + j) * N
                        nc.tensor.matmul(
                            psA[:128, j * H:(j + 1) * H],
                            slab[:, moff:moff + 128],
                            params_sb[:],
                            start=True, stop=True,
                        )
                        nc.tensor.matmul(
                            psB[:KT1, j * H:(j + 1) * H],
                            slab[:, moff + 128:moff + N],
                            params_sb[:],
                            start=True, stop=True,
                        )
                    qp0 = s * QP_SLAB + g0
                    nc.vector.tensor_copy(
                        TR_lo[:, 1 + qp0:1 + qp0 + glen, :],
                        psA[:128, :glen * H],
                    )
                    nc.vector.tensor_copy(
                        TR_hi[0:KT1, 1 + qp0:1 + qp0 + glen, :],
                        psB[:KT1, :glen * H],
                    )

        # ---------------- phase T: xT build ----------------
        xT = persist.tile([128, CK, tok], F32)
        with tc.tile_pool(name="xslab", bufs=3) as xslab:
            for mo in range(n_mo):
                cnt = min(128, tok - mo * 128)
                xs = xslab.tile([128, C], F32, tag="xs")
                nc.sync.dma_start(xs[:cnt, :], io["x"][mo * 128:mo * 128 + cnt, :])
                for kc in range(CK):
                    pst = psmall.tile([128, 512], F32, tag="ps")
                    nc.tensor.transpose(
                        pst[:128, :cnt],
                        xs[:cnt, kc * 128:(kc + 1) * 128],
                        ident[:cnt, :cnt],
                    )
                    nc.vector.tensor_copy(
                        xT[:, kc, mo * 128:mo * 128 + cnt], pst[:128, :cnt]
                    )

        # ---------------- per-b fused loop ----------------
        for b in range(nb):
            t0 = b * N
            # --- QKV: q and v in Form A ([tok, d], tok on partitions) ---
            q_sb = work1.tile([128, 2, C], F32, tag="q_sb")
            v_sb = work1.tile([128, 2, H, D + 1], F32, tag="v_sb")
            nc.vector.memset(v_sb[:, :, :, D:D + 1], 1.0)
            for mt, (moff, cnt) in enumerate(((0, KT0), (KT0, KT1))):
                for half, col0 in ((0, 0), (1, 2 * C)):   # q cols, v cols
                    psqv = pbig.tile([128, 2, 512], F32, tag="ps2")
                    for kc in range(CK):
                        for j in range(2):
                            nc.tensor.matmul(
                                psqv[:cnt, j, :384],
                                xT[:, kc, t0 + moff:t0 + moff + cnt],
                                w_sb[:, kc, col0 + j * 384:col0 + (j + 1) * 384],
                                start=(kc == 0), stop=(kc == CK - 1),
                            )
                    if half == 0:
                        for j in range(2):
                            nc.vector.tensor_tensor(
                                q_sb[:cnt, mt, j * 384:(j + 1) * 384],
                                psqv[:cnt, j, :384],
                                qkvb_rep[:cnt, j * 384:(j + 1) * 384],
                                mybir.AluOpType.add,
                            )
                    else:
                        for h in range(H):
                            j, off = divmod(h * D, 384)
                            nc.vector.tensor_tensor(
                                v_sb[:cnt, mt, h, 0:D],
                                psqv[:cnt, j, off:off + D],
                                qkvb_rep[:cnt, 2 * C + h * D:2 * C + (h + 1) * D],
                                mybir.AluOpType.add,
                            )
            # --- QKV: k in Form B ([d, tok], d on partitions) ---
            kT_sb = work1.tile([128, CK, N], F32, tag="kT_sb")
            for nk in range(CK):
                psk = psmall.tile([128, 512], F32, tag="ps")
                for kc in range(CK):
                    nc.tensor.matmul(
                        psk[:, :N],
                        w_sb[:, kc, C + nk * 128:C + (nk + 1) * 128],
                        xT[:, kc, t0:t0 + N],
                        start=(kc == 0), stop=(kc == CK - 1),
                    )
                nc.vector.tensor_scalar_add(
                    kT_sb[:, nk, :], psk[:, :N], qkvb_k[:, nk:nk + 1]
                )

            # --- attention per head ---
            otT = work2.tile([128, CK, N], F32, tag="otT")
            for h in range(H):
                # SAVE: q'^T [d, q] = sum_k q_raw[k, d] * TF[k, q]
                # For odd heads place the result at partition base 64 so the
                # scores matmul operand bases match kT's head slice.
                kh_p = (h % 2) * D
                kh_c = h // 2
                psqp = psmall.tile([128, 512], F32, tag="ps")
                nc.tensor.matmul(
                    psqp[kh_p:kh_p + D, :N],
                    q_sb[0:KT0, 0, h * D:(h + 1) * D],
                    TR_lo[:, :, h],
                    start=True, stop=False,
                )
                nc.tensor.matmul(
                    psqp[kh_p:kh_p + D, :N],
                    q_sb[0:KT1, 1, h * D:(h + 1) * D],
                    TR_hi[:, :, h],
                    start=False, stop=True,
                )
                qpT = work3.tile([128, N], F32, tag="qpT")
                nc.vector.tensor_copy(
                    qpT[kh_p:kh_p + D, :], psqp[kh_p:kh_p + D, :N]
                )

                # scores S^T [kt, qt] and exp
                pss0 = psmall.tile([128, 512], F32, tag="ps")
                pss1 = psmall.tile([128, 512], F32, tag="ps")
                nc.tensor.matmul(
                    pss0[:KT0, :N],
                    kT_sb[kh_p:kh_p + D, kh_c, 0:KT0],
                    qpT[kh_p:kh_p + D, :],
                    start=True, stop=True,
                )
                nc.tensor.matmul(
                    pss1[:KT1, :N],
                    kT_sb[kh_p:kh_p + D, kh_c, KT0:N],
                    qpT[kh_p:kh_p + D, :],
                    start=True, stop=True,
                )
                ex0 = work2.tile([KT0, N], F32, tag="ex0")
                ex1 = work2.tile([KT1, N], F32, tag="ex1")
                nc.scalar.activation(
                    ex0[:], pss0[:KT0, :N],
                    mybir.ActivationFunctionType.Exp, bias=0.0, scale=SCALE,
                )
                nc.scalar.activation(
                    ex1[:], pss1[:KT1, :N],
                    mybir.ActivationFunctionType.Exp, bias=0.0, scale=SCALE,
                )

                # PV with ones-augmented V: rows 0:64 = unnormalized out^T,
                # row 64 = softmax denominator
                pso = psmall.tile([128, 512], F32, tag="ps")
                nc.tensor.matmul(
                    pso[:D + 1, :N], v_sb[0:KT0, 0, h, :], ex0[:],
                    start=True, stop=False,
                )
                nc.tensor.matmul(
                    pso[:D + 1, :N], v_sb[0:KT1, 1, h, :], ex1[:],
                    start=False, stop=True,
                )
                rec = work3.tile([1, N], F32, tag="rec")
                nc.vector.reciprocal(rec[:], pso[D:D + 1, :N])
                psrep = psmall.tile([128, 512], F32, tag="ps")
                nc.tensor.matmul(
                    psrep[:D, :N], ones_sb[:], rec[:], start=True, stop=True
                )
                rep_sb = work3.tile([D, N], F32, tag="rep_sb")
                nc.vector.tensor_copy(rep_sb[:], psrep[:D, :N])
                nc.vector.tensor_tensor(
                    otT[kh_p:kh_p + D, kh_c, :],
                    pso[:D, :N],
                    rep_sb[:],
                    mybir.AluOpType.mult,
                )

            # --- output projection ---
            for mt, (moff, cnt) in enumerate(((0, KT0), (KT0, KT1))):
                psy = pbig.tile([128, 2, 512], F32, tag="ps2")
                for kc in range(CK):
                    for j in range(2):
                        nc.tensor.matmul(
                            psy[:cnt, j, :384],
                            otT[:, kc, moff:moff + cnt],
                            pw_sb[:, kc, j * 384:(j + 1) * 384],
                            start=(kc == 0), stop=(kc == CK - 1),
                        )
                y_sb = work2.tile([128, C], F32, tag="y_sb")
                for j in range(2):
                    nc.vector.tensor_tensor(
                        y_sb[:cnt, j * 384:(j + 1) * 384],
                        psy[:cnt, j, :384],
                        projb_rep[:cnt, j * 384:(j + 1) * 384],
                        mybir.AluOpType.add,
                    )
                nc.sync.dma_start(
                    io["y"][t0 + moff:t0 + moff + cnt, :], y_sb[:cnt, :]
                )


def build_nc(nb=NB):
    tok = nb * N
    nc = bacc.Bacc("TRN2", target_bir_lowering=False, debug=False)
    io = {
        "x": nc.dram_tensor("x", [tok, C], F32, kind="ExternalInput").ap(),
        "qkv_w": nc.dram_tensor("qkv_w", [C, 3 * C], F32, kind="ExternalInput").ap(),
        "qkv_b": nc.dram_tensor("qkv_b", [3 * C], F32, kind="ExternalInput").ap(),
        "proj_w": nc.dram_tensor("proj_w", [C, C], F32, kind="ExternalInput").ap(),
        "proj_b": nc.dram_tensor("proj_b", [C], F32, kind="ExternalInput").ap(),
        "tabT_aug": nc.dram_tensor(
            "tabT_aug", [WA, L * N], F32, kind="ExternalInput"
        ).ap(),
        "params_aug": nc.dram_tensor(
            "params_aug", [WA, H], F32, kind="ExternalInput"
        ).ap(),
        "ident": nc.dram_tensor("ident", [128, 128], F32, kind="ExternalInput").ap(),
        "y": nc.dram_tensor("y", [tok, C], F32, kind="ExternalOutput").ap(),
    }
    with tile.TileContext(nc) as tc:
        _body(tc, io, nb)
    nc.compile()
    return nc


def host_prep(inputs):
    """Host-side input prep shared by all cores (layout only, no math)."""
    # [WA, L, N]: per-qp blocks; col 0 zero-pad (cls row), cols 1:197 = the
    # table transposed to w-on-partitions, plus an identity-indicator row.
    tabT_aug = np.zeros((WA, L, N), np.float32)
    tabT_aug[:W, :, 1:] = inputs["spatial_table"].astype(np.float32).transpose(2, 0, 1)
    tabT_aug[W, :, 1:] = np.eye(L, dtype=np.float32)
    tabT_aug = tabT_aug.reshape(WA, L * N)
    params_aug = np.concatenate(
        [inputs["spatial_params"].astype(np.float32), np.ones((1, H), np.float32)], 0
    )
    return {
        "qkv_w": np.ascontiguousarray(inputs["qkv_w"], np.float32),
        "qkv_b": np.ascontiguousarray(inputs["qkv_b"], np.float32),
        "proj_w": np.ascontiguousarray(inputs["proj_w"], np.float32),
        "proj_b": np.ascontiguousarray(inputs["proj_b"], np.float32),
        "tabT_aug": tabT_aug,
        "params_aug": params_aug,
        "ident": np.eye(128, dtype=np.float32),
    }


_NC_CACHE = {}


def kernel(x, qkv_w, qkv_b, proj_w, proj_b, spatial_table, spatial_params,
           trace=False, trace_kwargs=None, tmpdir=None):
    inputs = dict(x=np.asarray(x), qkv_w=np.asarray(qkv_w),
                  qkv_b=np.asarray(qkv_b), proj_w=np.asarray(proj_w),
                  proj_b=np.asarray(proj_b),
                  spatial_table=np.asarray(spatial_table),
                  spatial_params=np.asarray(spatial_params))
    shared = host_prep(inputs)
    xs = inputs["x"].astype(np.float32).reshape(B, N, C)

    if NB not in _NC_CACHE:
        _NC_CACHE[NB] = build_nc(NB)
    nc = _NC_CACHE[NB]

    in_maps = []
    for i in range(NCORES):
        m = dict(shared)
        m["x"] = np.ascontiguousarray(
            xs[i * NB:(i + 1) * NB].reshape(NB * N, C)
        )
        in_maps.append(m)

    kw = {}
    if trace:
        kw = dict(trace=True, trace_kwargs=trace_kwargs or {}, tmpdir=tmpdir)
    res = run_bass_kernel_spmd(nc, in_maps, list(range(NCORES)), **kw)
    y = np.concatenate(
        [res.results[i]["y"].reshape(NB, N, C) for i in range(NCORES)], 0
    )
    if trace:
        return y, res
    return y
